# revision 3
# baseline (speedup 1.0000x reference)
"""MoE expert-MLP (SwiGLU) kernel for 8 Trainium2 NeuronCores.

Strategy: expert-parallel. Each of the 8 cores owns one expert's weights.
Tokens are routed on the host: every (token, k) routing slot is dispatched to
its expert's core, padded to a fixed per-expert capacity. Each core runs a
dense [cap, D] SwiGLU MLP for its expert in fp32r (full-rate fp32 matmul
mode on the PE array) and scales rows by the routing weight. The host then
scatter-combines the K=2 per-token contributions. No collectives needed.

Per-core kernel (cap tokens, D=2048, H=1408), loops in passes of Tc=768
tokens so x^T, h^T stay SBUF-resident per pass while Wg/Wu stream per
h-tile. Wd stays fully resident.
  stage A: h^T[h, t] = silu(Wg @ x^T) * (Wu @ x^T)   (PSUM accumulate over D)
  stage B: y[t, d]  = (h^T)^T @ Wd^T, row-scaled by routing weight
"""

import sys
import os

sys.path.insert(0, "/opt/trn_rl_repo")

import numpy as np

T, D, H, E, K = 8192, 2048, 1408, 8, 2
P = 128
TC = 768           # tokens per pass
TG = 384           # moving-dim group for stage A (>=256 keeps f32r full rate)
HT = H // P        # 11 h-tiles
KT = D // P        # 16 d-tiles
DC = 512           # moving-dim chunk for stage B

_built = {}


def _build_nc(cap):
    import concourse.bass as bass  # noqa: F401
    from concourse import bacc
    import concourse.mybir as mybir
    import concourse.tile as tile

    F32 = mybir.dt.float32
    F32R = mybir.dt.float32r
    Silu = mybir.ActivationFunctionType.Silu
    Mult = mybir.AluOpType.mult

    passes = cap // TC
    assert cap % TC == 0

    nc = bacc.Bacc("TRN2", target_bir_lowering=False, debug=False)
    xT = nc.declare_dram_parameter("xT", [D, cap], F32R, isOutput=False)
    wg = nc.declare_dram_parameter("wg", [HT, P, KT * P], F32R, isOutput=False)
    wu = nc.declare_dram_parameter("wu", [HT, P, KT * P], F32R, isOutput=False)
    wd = nc.declare_dram_parameter("wd", [H, D], F32R, isOutput=False)
    wt = nc.declare_dram_parameter("wt", [cap], F32, isOutput=False)
    out = nc.declare_dram_parameter("out", [cap, D], F32, isOutput=True)

    with tile.TileContext(nc) as tc:
        with (
            tc.tile_pool(name="sbuf", bufs=1) as pool,
            tc.tile_pool(name="psum", bufs=1, space="PSUM") as pp,
        ):
            # Wd resident: [128, 11, 2048] (88KB/partition)
            wd_t = pool.tile([P, HT, D], F32R, tag="wd", bufs=1)
            nc.sync.dma_start(wd_t[:], wd.rearrange("(ht p) d -> p ht d", p=P))
            # routing weights, one column per 128-token t_sub
            wt_t = pool.tile([P, cap // P], F32, tag="wt", bufs=1)
            nc.sync.dma_start(wt_t[:], wt.rearrange("(n p) -> p n", p=P))

            for pi in range(passes):
                t0 = pi * TC
                # x^T for this pass: [128, 16, 768] (48KB/partition)
                xt_t = pool.tile([P, KT, TC], F32R, tag="xt", bufs=1)
                nc.sync.dma_start(
                    xt_t[:],
                    xT[:, t0 : t0 + TC].rearrange("(k p) t -> p k t", p=P),
                )
                # h^T for this pass: [128, 11, 768] (33KB/partition)
                h_t = pool.tile([P, HT, TC], F32R, tag="ht", bufs=1)

                # ---- stage A: h^T = silu(g^T) * u^T ----
                for ht in range(HT):
                    wg_t = pool.tile([P, KT * P], F32R, tag="wgu", bufs=3)
                    nc.sync.dma_start(wg_t[:], wg[ht, :, :])
                    wu_t = pool.tile([P, KT * P], F32R, tag="wgu", bufs=3)
                    nc.sync.dma_start(wu_t[:], wu[ht, :, :])

                    psg = [pp.tile([P, TG], F32, tag=f"g{i}", bufs=1, name=f"psg{i}") for i in range(2)]
                    psu = [pp.tile([P, TG], F32, tag=f"u{i}", bufs=1, name=f"psu{i}") for i in range(2)]
                    for d in range(KT):
                        lhs = wg_t[:, d * P : (d + 1) * P]
                        for tg in range(2):
                            nc.tensor.matmul(
                                psg[tg][:],
                                lhs,
                                xt_t[:, d, tg * TG : (tg + 1) * TG],
                                start=(d == 0),
                                stop=(d == KT - 1),
                            )
                    silu_ts = []
                    for tg in range(2):
                        st = pool.tile([P, TG], F32, tag="silu", bufs=2)
                        nc.scalar.activation(st[:], psg[tg][:], Silu)
                        silu_ts.append(st)
                    for d in range(KT):
                        lhs = wu_t[:, d * P : (d + 1) * P]
                        for tg in range(2):
                            nc.tensor.matmul(
                                psu[tg][:],
                                lhs,
                                xt_t[:, d, tg * TG : (tg + 1) * TG],
                                start=(d == 0),
                                stop=(d == KT - 1),
                            )
                    for tg in range(2):
                        nc.vector.tensor_tensor(
                            h_t[:, ht, tg * TG : (tg + 1) * TG],
                            silu_ts[tg][:],
                            psu[tg][:],
                            op=Mult,
                        )

                # ---- stage B: y = h @ Wd^T, scaled by routing weight ----
                for ts_ in range(TC // P):
                    psy = [pp.tile([P, DC], F32, tag=f"y{i}", bufs=1, name=f"psy{i}") for i in range(4)]
                    for ht in range(HT):
                        lhs = h_t[:, ht, ts_ * P : (ts_ + 1) * P]
                        for dc in range(4):
                            nc.tensor.matmul(
                                psy[dc][:],
                                lhs,
                                wd_t[:, ht, dc * DC : (dc + 1) * DC],
                                start=(ht == 0),
                                stop=(ht == HT - 1),
                            )
                    col = t0 // P + ts_
                    for half in range(2):
                        y_t = pool.tile([P, D // 2], F32, tag="yout", bufs=2, name="y_t")
                        for j in range(2):
                            dc = half * 2 + j
                            nc.vector.tensor_scalar_mul(
                                y_t[:, j * DC : (j + 1) * DC],
                                psy[dc][:],
                                wt_t[:, col : col + 1],
                            )
                        nc.sync.dma_start(
                            out[
                                t0 + ts_ * P : t0 + (ts_ + 1) * P,
                                half * (D // 2) : (half + 1) * (D // 2),
                            ],
                            y_t[:],
                        )

    nc.finalize()
    return nc


def _get_nc(cap):
    if cap not in _built:
        _built[cap] = _build_nc(cap)
    return _built[cap]


def kernel(x, weights, Wg, Wu, Wd, indices, seq_len=None, **_unused):
    from concourse.bass_utils import run_bass_kernel_spmd

    x = np.asarray(x, dtype=np.float32)
    weights = np.asarray(weights, dtype=np.float32)
    Wg = np.asarray(Wg, dtype=np.float32)
    Wu = np.asarray(Wu, dtype=np.float32)
    Wd = np.asarray(Wd, dtype=np.float32)
    indices = np.asarray(indices)

    t, d = x.shape
    e, h, _ = Wg.shape
    k = indices.shape[1]

    # ---- host-side routing (dispatch) ----
    flat_e = indices.reshape(-1).astype(np.int64)
    flat_w = weights.reshape(-1)
    flat_t = np.repeat(np.arange(t, dtype=np.int64), k)
    order = np.argsort(flat_e, kind="stable")
    counts = np.bincount(flat_e, minlength=e)
    starts = np.zeros(e + 1, dtype=np.int64)
    starts[1:] = np.cumsum(counts)
    cap = int(-(-max(int(counts.max()), 1) // TC) * TC)

    tok_sorted = flat_t[order]
    w_sorted = flat_w[order]

    in_maps = []
    for ei in range(e):
        n = int(counts[ei])
        toks = tok_sorted[starts[ei] : starts[ei] + n]
        xe = np.zeros((cap, d), dtype=np.float32)
        xe[:n] = x[toks]
        wvec = np.zeros(cap, dtype=np.float32)
        wvec[:n] = w_sorted[starts[ei] : starts[ei] + n]
        # pack Wg/Wu so each h-tile block is one contiguous [128, 2048] DMA:
        # block[ht][p][k*128+hh] = Wg[e].T[k*128+p, ht*128+hh]
        WgT = Wg[ei].T  # [D, H]
        WuT = Wu[ei].T
        wg_lin = np.ascontiguousarray(
            WgT.reshape(KT, P, HT, P).transpose(2, 1, 0, 3).reshape(HT, P, KT * P)
        )
        wu_lin = np.ascontiguousarray(
            WuT.reshape(KT, P, HT, P).transpose(2, 1, 0, 3).reshape(HT, P, KT * P)
        )
        wdT = np.ascontiguousarray(Wd[ei].T)  # [H, D]
        in_maps.append(
            {
                "xT": np.ascontiguousarray(xe.T),
                "wg": wg_lin,
                "wu": wu_lin,
                "wd": wdT,
                "wt": wvec,
            }
        )

    nc = _get_nc(cap)
    trace = bool(int(os.environ.get("KERNEL_TRACE", "0")))
    res = run_bass_kernel_spmd(
        nc, in_maps, core_ids=list(range(e)), trace=trace
    )
    if trace:
        kernel.last_exec_time_ns = res.exec_time_ns
        kernel.last_results = res

    # ---- host-side combine ----
    allres = np.concatenate(
        [res.results[ei]["out"][: counts[ei]] for ei in range(e)], axis=0
    )
    inv = np.empty(t * k, dtype=np.int64)
    inv[order] = np.arange(t * k, dtype=np.int64)
    y = allres[inv].reshape(t, k, d).sum(axis=1, dtype=np.float32)
    return y


# revision 4
# speedup vs baseline: 1.0649x; 1.0649x over previous
"""MoE expert-MLP (SwiGLU) kernel for 8 Trainium2 NeuronCores.

Strategy: expert-parallel. Each of the 8 cores owns one expert's weights.
Tokens are routed on the host: every (token, k) routing slot is dispatched to
its expert's core, padded to a fixed per-expert capacity. Each core runs a
dense [cap, D] SwiGLU MLP for its expert in fp32r (full-rate fp32 matmul
mode on the PE array) and scales rows by the routing weight. The host then
scatter-combines the K=2 per-token contributions. No collectives needed.

Per-core kernel (cap tokens, D=2048, H=1408), loops in passes of <=768
tokens so x^T, h^T stay SBUF-resident per pass while Wg/Wu stream per
h-tile. Wd stays fully resident. The first pass is smaller so the PE can
start as soon as the first x^T d-chunk lands.
  stage A: h^T[h, t] = silu(Wg @ x^T) * (Wu @ x^T)   (PSUM accumulate over D)
  stage B: y[t, d]  = (h^T)^T @ Wd^T, row-scaled by routing weight
"""

import sys
import os

sys.path.insert(0, "/opt/trn_rl_repo")

import numpy as np

T, D, H, E, K = 8192, 2048, 1408, 8, 2
P = 128
HT = H // P        # 11 h-tiles
KT = D // P        # 16 d-tiles
DC = 512           # moving-dim chunk for stage B

_built = {}


def _pass_sizes(cap):
    """Split cap into passes: first ~640 (fast startup), rest 768.
    Every size is a multiple of 128 in [256, 768]."""
    sizes = []
    rem = cap
    first = 640 if rem >= 640 + 256 else rem
    sizes.append(first)
    rem -= first
    while rem:
        if rem <= 768:
            s = rem
        elif rem - 768 >= 256:
            s = 768
        else:
            s = rem - 256
        sizes.append(s)
        rem -= s
    assert all(s % 128 == 0 and 256 <= s <= 768 for s in sizes), sizes
    return sizes


def _tg_split(s):
    """Split a pass into <=2 moving-dim groups, each in [256, 512]."""
    if s <= 512:
        return [s]
    return [s - 384, 384]


def _build_nc(cap):
    import concourse.bass as bass  # noqa: F401
    from concourse import bacc
    import concourse.mybir as mybir
    import concourse.tile as tile

    F32 = mybir.dt.float32
    F32R = mybir.dt.float32r
    Silu = mybir.ActivationFunctionType.Silu
    Mult = mybir.AluOpType.mult

    sizes = _pass_sizes(cap)

    nc = bacc.Bacc("TRN2", target_bir_lowering=False, debug=False)
    xT = nc.declare_dram_parameter("xT", [D, cap], F32R, isOutput=False)
    wg = nc.declare_dram_parameter("wg", [HT, P, KT * P], F32R, isOutput=False)
    wu = nc.declare_dram_parameter("wu", [HT, P, KT * P], F32R, isOutput=False)
    wd = nc.declare_dram_parameter("wd", [H, D], F32R, isOutput=False)
    wt = nc.declare_dram_parameter("wt", [cap], F32, isOutput=False)
    out = nc.declare_dram_parameter("out", [cap, D], F32, isOutput=True)

    with tile.TileContext(nc) as tc:
        with (
            tc.tile_pool(name="sbuf", bufs=1) as pool,
            tc.tile_pool(name="psum", bufs=1, space="PSUM") as pp,
        ):
            wd_t = None
            wt_t = None
            t0 = 0
            for pi, TC in enumerate(sizes):
                tgs = _tg_split(TC)
                # x^T for this pass, one tile per d-tile so the first
                # matmuls only wait on the first chunk's DMA
                xt_ts = []
                for dti in range(KT):
                    xt_1 = pool.tile([P, TC], F32R, tag=f"xt{dti}", bufs=1,
                                     name=f"xt{dti}")
                    nc.sync.dma_start(
                        xt_1[:], xT[dti * P : (dti + 1) * P, t0 : t0 + TC]
                    )
                    xt_ts.append(xt_1)
                # h^T for this pass
                h_t = pool.tile([P, HT, TC], F32R, tag="ht", bufs=1)

                # ---- stage A: h^T = silu(g^T) * u^T ----
                for ht in range(HT):
                    wg_t = pool.tile([P, KT * P], F32R, tag="wgu", bufs=3)
                    nc.sync.dma_start(wg_t[:], wg[ht, :, :])
                    wu_t = pool.tile([P, KT * P], F32R, tag="wgu", bufs=3)
                    nc.sync.dma_start(wu_t[:], wu[ht, :, :])

                    psg = [pp.tile([P, g], F32, tag=f"g{i}", bufs=1,
                                   name=f"psg{i}") for i, g in enumerate(tgs)]
                    psu = [pp.tile([P, g], F32, tag=f"u{i}", bufs=1,
                                   name=f"psu{i}") for i, g in enumerate(tgs)]
                    off = [0, tgs[0]]
                    for d in range(KT):
                        lhs = wg_t[:, d * P : (d + 1) * P]
                        for tg, g in enumerate(tgs):
                            nc.tensor.matmul(
                                psg[tg][:],
                                lhs,
                                xt_ts[d][:, off[tg] : off[tg] + g],
                                start=(d == 0),
                                stop=(d == KT - 1),
                            )
                    silu_ts = []
                    for tg, g in enumerate(tgs):
                        st = pool.tile([P, g], F32, tag="silu", bufs=2,
                                       name="st")
                        nc.scalar.activation(st[:], psg[tg][:], Silu)
                        silu_ts.append(st)
                    for d in range(KT):
                        lhs = wu_t[:, d * P : (d + 1) * P]
                        for tg, g in enumerate(tgs):
                            nc.tensor.matmul(
                                psu[tg][:],
                                lhs,
                                xt_ts[d][:, off[tg] : off[tg] + g],
                                start=(d == 0),
                                stop=(d == KT - 1),
                            )
                    for tg, g in enumerate(tgs):
                        nc.vector.tensor_tensor(
                            h_t[:, ht, off[tg] : off[tg] + g],
                            silu_ts[tg][:],
                            psu[tg][:],
                            op=Mult,
                        )

                if pi == 0:
                    # Wd resident [128, 11, 2048] (88KB/partition) + routing
                    # weights; loaded here so they don't delay pass 0's x^T
                    wd_t = pool.tile([P, HT, D], F32R, tag="wd", bufs=1)
                    nc.sync.dma_start(
                        wd_t[:], wd.rearrange("(ht p) d -> p ht d", p=P)
                    )
                    wt_t = pool.tile([P, cap // P], F32, tag="wt", bufs=1)
                    nc.sync.dma_start(wt_t[:], wt.rearrange("(n p) -> p n", p=P))

                # ---- stage B: y = h @ Wd^T, scaled by routing weight ----
                for ts_ in range(TC // P):
                    psy = [pp.tile([P, DC], F32, tag=f"y{i}", bufs=1,
                                   name=f"psy{i}") for i in range(4)]
                    for ht in range(HT):
                        lhs = h_t[:, ht, ts_ * P : (ts_ + 1) * P]
                        for dc in range(4):
                            nc.tensor.matmul(
                                psy[dc][:],
                                lhs,
                                wd_t[:, ht, dc * DC : (dc + 1) * DC],
                                start=(ht == 0),
                                stop=(ht == HT - 1),
                            )
                    col = t0 // P + ts_
                    for half in range(2):
                        y_t = pool.tile([P, D // 2], F32, tag="yout", bufs=2,
                                        name="y_t")
                        for j in range(2):
                            dc = half * 2 + j
                            nc.vector.tensor_scalar_mul(
                                y_t[:, j * DC : (j + 1) * DC],
                                psy[dc][:],
                                wt_t[:, col : col + 1],
                            )
                        nc.sync.dma_start(
                            out[
                                t0 + ts_ * P : t0 + (ts_ + 1) * P,
                                half * (D // 2) : (half + 1) * (D // 2),
                            ],
                            y_t[:],
                        )
                t0 += TC

    nc.finalize()
    return nc


def _get_nc(cap):
    if cap not in _built:
        _built[cap] = _build_nc(cap)
    return _built[cap]


def kernel(x, weights, Wg, Wu, Wd, indices, seq_len=None, **_unused):
    from concourse.bass_utils import run_bass_kernel_spmd

    x = np.asarray(x, dtype=np.float32)
    weights = np.asarray(weights, dtype=np.float32)
    Wg = np.asarray(Wg, dtype=np.float32)
    Wu = np.asarray(Wu, dtype=np.float32)
    Wd = np.asarray(Wd, dtype=np.float32)
    indices = np.asarray(indices)

    t, d = x.shape
    e, h, _ = Wg.shape
    k = indices.shape[1]

    # ---- host-side routing (dispatch) ----
    flat_e = indices.reshape(-1).astype(np.int64)
    flat_w = weights.reshape(-1)
    flat_t = np.repeat(np.arange(t, dtype=np.int64), k)
    order = np.argsort(flat_e, kind="stable")
    counts = np.bincount(flat_e, minlength=e)
    starts = np.zeros(e + 1, dtype=np.int64)
    starts[1:] = np.cumsum(counts)
    cap = int(-(-max(int(counts.max()), 512) // P) * P)

    tok_sorted = flat_t[order]
    w_sorted = flat_w[order]

    in_maps = []
    for ei in range(e):
        n = int(counts[ei])
        toks = tok_sorted[starts[ei] : starts[ei] + n]
        xe = np.zeros((cap, d), dtype=np.float32)
        xe[:n] = x[toks]
        wvec = np.zeros(cap, dtype=np.float32)
        wvec[:n] = w_sorted[starts[ei] : starts[ei] + n]
        # pack Wg/Wu so each h-tile block is one contiguous [128, 2048] DMA:
        # block[ht][p][k*128+hh] = Wg[e].T[k*128+p, ht*128+hh]
        WgT = Wg[ei].T  # [D, H]
        WuT = Wu[ei].T
        wg_lin = np.ascontiguousarray(
            WgT.reshape(KT, P, HT, P).transpose(2, 1, 0, 3).reshape(HT, P, KT * P)
        )
        wu_lin = np.ascontiguousarray(
            WuT.reshape(KT, P, HT, P).transpose(2, 1, 0, 3).reshape(HT, P, KT * P)
        )
        wdT = np.ascontiguousarray(Wd[ei].T)  # [H, D]
        in_maps.append(
            {
                "xT": np.ascontiguousarray(xe.T),
                "wg": wg_lin,
                "wu": wu_lin,
                "wd": wdT,
                "wt": wvec,
            }
        )

    nc = _get_nc(cap)
    trace = bool(int(os.environ.get("KERNEL_TRACE", "0")))
    res = run_bass_kernel_spmd(
        nc, in_maps, core_ids=list(range(e)), trace=trace
    )
    if trace:
        kernel.last_exec_time_ns = res.exec_time_ns
        kernel.last_results = res

    # ---- host-side combine ----
    allres = np.concatenate(
        [res.results[ei]["out"][: counts[ei]] for ei in range(e)], axis=0
    )
    inv = np.empty(t * k, dtype=np.int64)
    inv[order] = np.arange(t * k, dtype=np.int64)
    y = allres[inv].reshape(t, k, d).sum(axis=1, dtype=np.float32)
    return y
